# revision 7
# baseline (speedup 1.0000x reference)
"""Trainium2 Bass kernel for AxonalConnections message passing.

Computes out[b, t] = sum_s spikes[b, s] * adjacency[t, s]
  spikes_A: [8, 128, 128] f32  -> flat [B=8, S=16384]
  adjacency: [16384, 16384] f32
  out: [8, 128, 128] f32

The stride adjacency is extremely sparse: only rows/columns touched by an
edge are nonzero (1024 of 16384 each for the reference's stride-4 grid).
out[:, t] is identically zero for any all-zero row t, and all-zero columns
s contribute nothing to the contraction.  Three device paths, picked by
runtime inspection of the adjacency (host does layout/gather only; every
FLOP on tensor values happens on device):

1. diag path: every nonzero row has exactly one nonzero entry (true for
   the reference's stride grid, where src_idx == tgt_idx), so
   out[b, t] = flat[b, s(t)] * w(t).  Host gathers the spike columns and
   the weights; the nonzero rows are sharded across the 8 cores; each
   core runs one DVE elementwise multiply over its row-block (raw bacc,
   no TileContext -- the whole body is DMA-in, multiply, DMA-out, ~3 us;
   measured total is dominated by the fixed NEFF preamble/epilogue).
2. block path: general small support.  Each core runs the full fp32
   contraction xsub @ Asub_m.T on the PE over the gathered nonzero
   rows/cols (Tile kernel, 8 accumulating matmuls per core).
3. dense fallback (the original full-stream kernel) for large support:
   streams the whole 1 GiB adjacency at the HBM roofline using an exact
   fp16 hi/lo split.
"""

import sys

if "/opt/trn_rl_repo" not in sys.path:
    sys.path.insert(0, "/opt/trn_rl_repo")

from concurrent.futures import ThreadPoolExecutor

import numpy as np

N_CORES = 8
B = 8
S = 16384            # source neurons (contraction dim)
T = 16384            # target neurons
P = 128              # partitions
TBLK = T // N_CORES  # 2048 targets per core (dense path)
S_TILES = S // P     # 128 stripes of the contraction dim (dense path)
G = 8                # s-stripes per DMA slab (dense path)
TCH_DENSE = 512
NCH_DENSE = TBLK // TCH_DENSE

# sparse-path limits: above these, fall back to the dense streamer
SPARSE_MAX_KC = 8192   # padded contraction size
SPARSE_MAX_TC = 2048   # padded per-core target-block size

_prog_cache = {}


# --------------------------------------------------------------------------
# diagonal-like path: every nonzero row of the adjacency has exactly one
# nonzero entry, so out[b, t] = flat[b, s(t)] * w(t).  Host gathers the
# spike columns and weights (pure layout); the device does the multiply.
# Raw bacc (no TileContext) keeps the fixed overhead minimal.
# --------------------------------------------------------------------------

def _build_diag_program(npart, free):
    import concourse.bacc as bacc
    from concourse import mybir

    f32 = mybir.dt.float32
    nc = bacc.Bacc("TRN2", target_bir_lowering=False, debug=False)
    # the flat [B, tc_pad] problem reshaped to [npart, free] for partition
    # parallelism; cols 0..free-1: gathered spikes, free..2*free-1: weights
    gw = nc.dram_tensor("gw", [npart, 2 * free], f32, kind="ExternalInput").ap()
    y = nc.dram_tensor("y", [npart, free], f32, kind="ExternalOutput").ap()

    with (
        nc.sbuf_tensor("t_sb", [npart, 2 * free], f32) as t_sb,
        nc.sbuf_tensor("y_sb", [npart, free], f32) as y_sb,
        nc.semaphore() as sem,
    ):
        nc.sync.dma_start(t_sb.ap(), gw).then_inc(sem, 16)
        nc.vector.wait_ge(sem, 16)
        nc.vector.tensor_tensor(
            y_sb.ap(),
            t_sb.ap()[:, 0:free],
            t_sb.ap()[:, free : 2 * free],
            op=mybir.AluOpType.mult,
        ).then_inc(sem, 1)
        nc.sync.wait_ge(sem, 17)
        nc.sync.dma_start(y, y_sb.ap()).then_inc(sem, 16)

    nc.compile()
    return nc


def _get_diag_program(npart, free):
    key = ("diag", npart, free)
    if key not in _prog_cache:
        _prog_cache[key] = _build_diag_program(npart, free)
    return _prog_cache[key]


def _diag_run(flat, rows, src, w, trace):
    """rows: nonzero target rows; src[i]: the single source col for rows[i];
    w[i]: the weight.  out[:, rows[i]] = flat[:, src[i]] * w[i]."""
    from concourse.bass_utils import run_bass_kernel_spmd

    nr = len(rows)
    per_core = -(-nr // N_CORES)
    tc_pad = max(8, -(-per_core // 8) * 8)
    # spread the [B, tc_pad] element-wise problem across more partitions
    import os
    npart = int(os.environ.get("DIAG_NPART", "64")) if tc_pad % 8 == 0 else B
    free = B * tc_pad // npart
    prog = _get_diag_program(npart, free)

    in_maps = []
    counts = []
    for m in range(N_CORES):
        lo = m * per_core
        hi = min(nr, lo + per_core)
        k = max(0, hi - lo)
        counts.append(k)
        g2 = np.zeros((B, tc_pad), np.float32)
        w2 = np.zeros((B, tc_pad), np.float32)
        if k:
            g2[:, :k] = flat[:, src[lo:hi]]
            w2[:, :k] = w[lo:hi][None, :]
        gw = np.concatenate(
            [g2.reshape(npart, free), w2.reshape(npart, free)], axis=1
        )
        in_maps.append({"gw": np.ascontiguousarray(gw)})

    res = run_bass_kernel_spmd(prog, in_maps, core_ids=list(range(N_CORES)), trace=trace)

    out = np.zeros((B, T), np.float32)
    for m in range(N_CORES):
        k = counts[m]
        if k:
            y2 = res.results[m]["y"].reshape(B, tc_pad)
            out[:, rows[m * per_core : m * per_core + k]] = y2[:, :k]
    return out.reshape(B, 128, 128), res


# --------------------------------------------------------------------------
# sparse path: dense sub-matmul over the nonzero rows/cols of the adjacency
# --------------------------------------------------------------------------

def _build_sparse_program(kc_pad, tc_pad):
    import concourse.tile as tile
    import concourse.bacc as bacc
    from concourse import bass, mybir

    f32 = mybir.dt.float32
    nstripe = kc_pad // P
    tch = 512 if tc_pad % 512 == 0 else 128
    nch = tc_pad // tch

    nc = bacc.Bacc("TRN2", target_bir_lowering=False, debug=False)
    # per-core adjacency block, pre-transposed: asb[s', t'] = A[nzr_m[t'], nzc[s']]
    asb = nc.dram_tensor("asb", [kc_pad, tc_pad], f32, kind="ExternalInput").ap()
    # stationary spikes: xs[p, i*B + b] = xsub[b, i*128 + p]
    xs = nc.dram_tensor("xs", [P, nstripe * B], f32, kind="ExternalInput").ap()
    y = nc.dram_tensor("y", [B, tc_pad], f32, kind="ExternalOutput").ap()

    with tile.TileContext(nc) as tc:
        with (
            tc.tile_pool(name="misc", bufs=1) as misc_pool,
            tc.tile_pool(name="psum", bufs=1, space=bass.MemorySpace.PSUM) as psum_pool,
        ):
            xs_sb = misc_pool.tile([P, nstripe * B], f32)
            nc.sync.dma_start(xs_sb[:], xs[:])
            at = misc_pool.tile([P, nstripe, tc_pad], f32)
            asb_r = asb.rearrange("(i p) t -> p i t", p=P)
            # per-stripe DMAs so matmul i only waits for stripe i
            for i in range(nstripe):
                nc.sync.dma_start(at[:, i : i + 1], asb_r[:, i : i + 1])
            y_sb = misc_pool.tile([B, tc_pad], f32)
            psums = [
                psum_pool.tile([B, tch], f32, name=f"psum{j}") for j in range(nch)
            ]
            for i in range(nstripe):
                lhsT = xs_sb[:, i * B : (i + 1) * B]
                for j in range(nch):
                    nc.tensor.matmul(
                        psums[j][:],
                        lhsT,
                        at[:, i, j * tch : (j + 1) * tch],
                        start=(i == 0),
                        stop=(i == nstripe - 1),
                    )
            for j in range(nch):
                nc.vector.tensor_copy(y_sb[:, j * tch : (j + 1) * tch], psums[j][:])
            nc.sync.dma_start(y[:], y_sb[:])

    nc.compile()
    return nc


def _get_sparse_program(kc_pad, tc_pad):
    key = ("sparse", kc_pad, tc_pad)
    if key not in _prog_cache:
        _prog_cache[key] = _build_sparse_program(kc_pad, tc_pad)
    return _prog_cache[key]


def _sparse_run(flat, adj, rows, cols, trace):
    from concourse.bass_utils import run_bass_kernel_spmd

    nr, kc = len(rows), len(cols)
    per_core = -(-nr // N_CORES) if nr else 0
    tc_pad = max(P, -(-per_core // P) * P)
    kc_pad = max(P, -(-kc // P) * P)
    nstripe = kc_pad // P
    prog = _get_sparse_program(kc_pad, tc_pad)

    xsub = np.zeros((B, kc_pad), np.float32)
    if kc:
        xsub[:, :kc] = flat[:, cols]
    xs_host = np.ascontiguousarray(
        xsub.reshape(B, nstripe, P).transpose(2, 1, 0).reshape(P, nstripe * B)
    )

    in_maps = []
    row_blocks = []
    for m in range(N_CORES):
        rows_m = rows[m * per_core : (m + 1) * per_core] if per_core else rows[:0]
        row_blocks.append(rows_m)
        asb_m = np.zeros((kc_pad, tc_pad), np.float32)
        if len(rows_m) and kc:
            asb_m[:kc, : len(rows_m)] = adj[np.ix_(rows_m, cols)].T
        in_maps.append({"asb": asb_m, "xs": xs_host})

    res = run_bass_kernel_spmd(prog, in_maps, core_ids=list(range(N_CORES)), trace=trace)

    out = np.zeros((B, T), np.float32)
    for m in range(N_CORES):
        rows_m = row_blocks[m]
        if len(rows_m):
            out[:, rows_m] = res.results[m]["y"][:, : len(rows_m)]
    return out.reshape(B, 128, 128), res


# --------------------------------------------------------------------------
# dense fallback: stream the full adjacency (fp16 hi/lo split, exact)
# --------------------------------------------------------------------------

def _build_dense_program():
    import concourse.bacc as bacc
    import concourse.tile as tile
    from concourse import bass, mybir

    f16 = mybir.dt.float16
    f32 = mybir.dt.float32

    nc = bacc.Bacc("TRN2", target_bir_lowering=False, debug=False)
    adjt2 = nc.dram_tensor("adjt2", [S, 2, TBLK], f16, kind="ExternalInput").ap()
    xt = nc.dram_tensor("xt", [P, S_TILES * 2 * B], f16, kind="ExternalInput").ap()
    # rows 0-7: xh*(ah+al); rows 8-15: xl*(ah+al); folded on the host
    y2 = nc.dram_tensor("y2", [2 * B, TBLK], f32, kind="ExternalOutput").ap()

    with tile.TileContext(nc) as tc:
        with (
            tc.tile_pool(name="adj", bufs=2) as adj_pool,
            tc.tile_pool(name="misc", bufs=1) as misc_pool,
            tc.tile_pool(name="psum", bufs=1, space=bass.MemorySpace.PSUM) as psum_pool,
        ):
            xt_sb = misc_pool.tile([P, S_TILES * 2 * B], f16)
            nc.sync.dma_start(xt_sb[:], xt[:])
            y_sb = misc_pool.tile([2 * B, TBLK], f32)
            psums = [
                psum_pool.tile([2 * B, TCH_DENSE], f32, name=f"psum{j}")
                for j in range(NCH_DENSE)
            ]

            # [S, 2, TBLK] -> [P, S_TILES, 2, TBLK]: stripe i on partition p
            adjt2_r = adjt2.rearrange("(i p) h t -> p i h t", p=P)
            slabs = [G] * (S_TILES // G)
            off = 0
            for si, sz in enumerate(slabs):
                at = adj_pool.tile([P, sz, 2, TBLK], f16, name="at", tag="at")
                if si == len(slabs) - 1:
                    # final slab: per-stripe sub-DMAs into the same slot, so the
                    # PE tail after the stream ends is one stripe, not eight.
                    for g in range(sz):
                        nc.sync.dma_start(
                            at[:, g : g + 1], adjt2_r[:, off + g : off + g + 1]
                        )
                elif si == len(slabs) - 2:
                    # half-slab deps let the PE start this slab mid-DMA
                    hs = sz // 2
                    nc.sync.dma_start(at[:, 0:hs], adjt2_r[:, off : off + hs])
                    nc.sync.dma_start(at[:, hs:sz], adjt2_r[:, off + hs : off + sz])
                else:
                    nc.sync.dma_start(at[:], adjt2_r[:, off : off + sz])
                for g in range(sz):
                    i = off + g
                    lhsT = xt_sb[:, i * 2 * B : (i + 1) * 2 * B]  # [xh | xl]
                    for j in range(NCH_DENSE):
                        for h in range(2):  # moving pass over a_hi then a_lo
                            nc.tensor.matmul(
                                psums[j][:],
                                lhsT,
                                at[:, g, h, j * TCH_DENSE : (j + 1) * TCH_DENSE],
                                start=(i == 0 and h == 0),
                                stop=(i == S_TILES - 1 and h == 1),
                            )
                off += sz
            assert off == S_TILES
            for j in range(NCH_DENSE):
                nc.vector.tensor_copy(
                    y_sb[:, j * TCH_DENSE : (j + 1) * TCH_DENSE], psums[j][:]
                )
            nc.sync.dma_start(y2[:], y_sb[:])

    nc.compile()
    return nc


def _get_dense_program():
    if "dense" not in _prog_cache:
        _prog_cache["dense"] = _build_dense_program()
    return _prog_cache["dense"]


def _split16(a32):
    hi = a32.astype(np.float16)
    lo = (a32 - hi.astype(np.float32)).astype(np.float16)
    return hi, lo


def _dense_host_prep(flat, adj):
    xh, xl = _split16(flat)
    # xt[p, i*16 + h*8 + b] = x_half[h][b, i*128 + p]
    arr = np.stack([xh.reshape(B, S_TILES, P), xl.reshape(B, S_TILES, P)], axis=0)
    xt_host = np.ascontiguousarray(
        arr.transpose(3, 2, 0, 1).reshape(P, S_TILES * 2 * B)
    )

    def prep_core(m):
        blkT = np.ascontiguousarray(adj[m * TBLK : (m + 1) * TBLK, :].T)  # [S, TBLK]
        ah, al = _split16(blkT)
        adjt2_m = np.ascontiguousarray(np.stack([ah, al], axis=1))  # [S, 2, TBLK]
        return {"adjt2": adjt2_m, "xt": xt_host}

    with ThreadPoolExecutor(max_workers=N_CORES) as ex:
        in_maps = list(ex.map(prep_core, range(N_CORES)))
    return in_maps


def _dense_run(flat, adj, trace):
    from concourse.bass_utils import run_bass_kernel_spmd

    nc = _get_dense_program()
    in_maps = _dense_host_prep(flat, adj)
    res = run_bass_kernel_spmd(nc, in_maps, core_ids=list(range(N_CORES)), trace=trace)
    out = np.concatenate(
        [res.results[m]["y2"][0:B] + res.results[m]["y2"][B : 2 * B]
         for m in range(N_CORES)],
        axis=1,
    )
    return out.reshape(B, 128, 128), res


# --------------------------------------------------------------------------
# entry points
# --------------------------------------------------------------------------

def run(spikes_A, adjacency, trace=False):
    """Run on hardware; returns (out [8,128,128] f32, BassKernelResults)."""
    flat = np.ascontiguousarray(np.asarray(spikes_A, dtype=np.float32)).reshape(B, S)
    adj = np.asarray(adjacency, dtype=np.float32)

    rows = np.flatnonzero(adj.any(axis=1))
    cols = np.flatnonzero(adj.any(axis=0))
    per_core = -(-len(rows) // N_CORES) if len(rows) else 0
    tc_pad = max(P, -(-per_core // P) * P)
    kc_pad = max(P, -(-len(cols) // P) * P)
    if kc_pad <= SPARSE_MAX_KC and tc_pad <= SPARSE_MAX_TC:
        if len(rows):
            # the nonzero sub-block is small; check for one-nonzero-per-row
            block = adj[np.ix_(rows, cols)]
            nz = block != 0
            if nz.sum(axis=1).max() == 1:
                cidx = np.argmax(nz, axis=1)
                w = block[np.arange(len(rows)), cidx]
                src = cols[cidx]
                return _diag_run(flat, rows, src, w, trace)
        return _sparse_run(flat, adj, rows, cols, trace)
    return _dense_run(flat, adj, trace)


def kernel(spikes_A, adjacency):
    out, _ = run(spikes_A, adjacency)
    return out


# revision 8
# speedup vs baseline: 1.0960x; 1.0960x over previous
"""Trainium2 Bass kernel for AxonalConnections message passing.

Computes out[b, t] = sum_s spikes[b, s] * adjacency[t, s]
  spikes_A: [8, 128, 128] f32  -> flat [B=8, S=16384]
  adjacency: [16384, 16384] f32
  out: [8, 128, 128] f32

The stride adjacency is extremely sparse: only rows/columns touched by an
edge are nonzero (1024 of 16384 each for the reference's stride-4 grid).
out[:, t] is identically zero for any all-zero row t, and all-zero columns
s contribute nothing to the contraction.  Three device paths, picked by
runtime inspection of the adjacency (host does layout/gather only; every
FLOP on tensor values happens on device):

1. diag path: every nonzero row has exactly one nonzero entry (true for
   the reference's stride grid, where src_idx == tgt_idx), so
   out[b, t] = flat[b, s(t)] * w(t).  Host gathers the spike columns and
   the weights; the nonzero rows are sharded across the 8 cores; each
   core runs one DVE elementwise multiply over its row-block (raw bacc,
   no TileContext -- the whole body is DMA-in, multiply, DMA-out, ~3 us;
   measured total is dominated by the fixed NEFF preamble/epilogue).
2. block path: general small support.  Each core runs the full fp32
   contraction xsub @ Asub_m.T on the PE over the gathered nonzero
   rows/cols (Tile kernel, 8 accumulating matmuls per core).
3. dense fallback (the original full-stream kernel) for large support:
   streams the whole 1 GiB adjacency at the HBM roofline using an exact
   fp16 hi/lo split.
"""

import sys

if "/opt/trn_rl_repo" not in sys.path:
    sys.path.insert(0, "/opt/trn_rl_repo")

from concurrent.futures import ThreadPoolExecutor

import numpy as np

N_CORES = 8
B = 8
S = 16384            # source neurons (contraction dim)
T = 16384            # target neurons
P = 128              # partitions
TBLK = T // N_CORES  # 2048 targets per core (dense path)
S_TILES = S // P     # 128 stripes of the contraction dim (dense path)
G = 8                # s-stripes per DMA slab (dense path)
TCH_DENSE = 512
NCH_DENSE = TBLK // TCH_DENSE

# sparse-path limits: above these, fall back to the dense streamer
SPARSE_MAX_KC = 8192   # padded contraction size
SPARSE_MAX_TC = 2048   # padded per-core target-block size

_prog_cache = {}


# --------------------------------------------------------------------------
# diagonal-like path: every nonzero row of the adjacency has exactly one
# nonzero entry, so out[b, t] = flat[b, s(t)] * w(t).  Host gathers the
# spike columns and weights (pure layout); the device does the multiply.
# Raw bacc (no TileContext) keeps the fixed overhead minimal.
# --------------------------------------------------------------------------

def _build_diag_program(npart, free):
    import concourse.bacc as bacc
    from concourse import mybir

    f32 = mybir.dt.float32
    nc = bacc.Bacc("TRN2", target_bir_lowering=False, debug=False)
    # the flat [B, tc_pad] problem reshaped to [npart, free] for partition
    # parallelism; cols 0..free-1: gathered spikes, free..2*free-1: weights
    gw = nc.dram_tensor("gw", [npart, 2 * free], f32, kind="ExternalInput").ap()
    y = nc.dram_tensor("y", [npart, free], f32, kind="ExternalOutput").ap()

    with (
        nc.sbuf_tensor("t_sb", [npart, 2 * free], f32) as t_sb,
        nc.sbuf_tensor("y_sb", [npart, free], f32) as y_sb,
        nc.semaphore() as sem,
    ):
        nc.sync.dma_start(t_sb.ap(), gw).then_inc(sem, 16)
        nc.vector.wait_ge(sem, 16)
        nc.vector.tensor_tensor(
            y_sb.ap(),
            t_sb.ap()[:, 0:free],
            t_sb.ap()[:, free : 2 * free],
            op=mybir.AluOpType.mult,
        ).then_inc(sem, 1)
        nc.sync.wait_ge(sem, 17)
        nc.sync.dma_start(y, y_sb.ap()).then_inc(sem, 16)

    nc.compile()
    return nc


def _get_diag_program(npart, free):
    key = ("diag", npart, free)
    if key not in _prog_cache:
        _prog_cache[key] = _build_diag_program(npart, free)
    return _prog_cache[key]


def _diag_run(flat, rows, src, w, trace):
    """rows: nonzero target rows; src[i]: the single source col for rows[i];
    w[i]: the weight.  out[:, rows[i]] = flat[:, src[i]] * w[i]."""
    from concourse.bass_utils import run_bass_kernel_spmd

    nr = len(rows)
    per_core = -(-nr // N_CORES)
    tc_pad = max(8, -(-per_core // 8) * 8)
    # spread the [B, tc_pad] element-wise problem across 64 partitions:
    # measured best on HW (DMA issue + DVE) vs 8/16/32/128-partition layouts
    npart = 64 if tc_pad % 8 == 0 else B
    free = B * tc_pad // npart
    prog = _get_diag_program(npart, free)

    in_maps = []
    counts = []
    for m in range(N_CORES):
        lo = m * per_core
        hi = min(nr, lo + per_core)
        k = max(0, hi - lo)
        counts.append(k)
        g2 = np.zeros((B, tc_pad), np.float32)
        w2 = np.zeros((B, tc_pad), np.float32)
        if k:
            g2[:, :k] = flat[:, src[lo:hi]]
            w2[:, :k] = w[lo:hi][None, :]
        gw = np.concatenate(
            [g2.reshape(npart, free), w2.reshape(npart, free)], axis=1
        )
        in_maps.append({"gw": np.ascontiguousarray(gw)})

    res = run_bass_kernel_spmd(prog, in_maps, core_ids=list(range(N_CORES)), trace=trace)

    out = np.zeros((B, T), np.float32)
    for m in range(N_CORES):
        k = counts[m]
        if k:
            y2 = res.results[m]["y"].reshape(B, tc_pad)
            out[:, rows[m * per_core : m * per_core + k]] = y2[:, :k]
    return out.reshape(B, 128, 128), res


# --------------------------------------------------------------------------
# sparse path: dense sub-matmul over the nonzero rows/cols of the adjacency
# --------------------------------------------------------------------------

def _build_sparse_program(kc_pad, tc_pad):
    import concourse.tile as tile
    import concourse.bacc as bacc
    from concourse import bass, mybir

    f32 = mybir.dt.float32
    nstripe = kc_pad // P
    tch = 512 if tc_pad % 512 == 0 else 128
    nch = tc_pad // tch

    nc = bacc.Bacc("TRN2", target_bir_lowering=False, debug=False)
    # per-core adjacency block, pre-transposed: asb[s', t'] = A[nzr_m[t'], nzc[s']]
    asb = nc.dram_tensor("asb", [kc_pad, tc_pad], f32, kind="ExternalInput").ap()
    # stationary spikes: xs[p, i*B + b] = xsub[b, i*128 + p]
    xs = nc.dram_tensor("xs", [P, nstripe * B], f32, kind="ExternalInput").ap()
    y = nc.dram_tensor("y", [B, tc_pad], f32, kind="ExternalOutput").ap()

    with tile.TileContext(nc) as tc:
        with (
            tc.tile_pool(name="misc", bufs=1) as misc_pool,
            tc.tile_pool(name="psum", bufs=1, space=bass.MemorySpace.PSUM) as psum_pool,
        ):
            xs_sb = misc_pool.tile([P, nstripe * B], f32)
            nc.sync.dma_start(xs_sb[:], xs[:])
            at = misc_pool.tile([P, nstripe, tc_pad], f32)
            asb_r = asb.rearrange("(i p) t -> p i t", p=P)
            # per-stripe DMAs so matmul i only waits for stripe i
            for i in range(nstripe):
                nc.sync.dma_start(at[:, i : i + 1], asb_r[:, i : i + 1])
            y_sb = misc_pool.tile([B, tc_pad], f32)
            psums = [
                psum_pool.tile([B, tch], f32, name=f"psum{j}") for j in range(nch)
            ]
            for i in range(nstripe):
                lhsT = xs_sb[:, i * B : (i + 1) * B]
                for j in range(nch):
                    nc.tensor.matmul(
                        psums[j][:],
                        lhsT,
                        at[:, i, j * tch : (j + 1) * tch],
                        start=(i == 0),
                        stop=(i == nstripe - 1),
                    )
            for j in range(nch):
                nc.vector.tensor_copy(y_sb[:, j * tch : (j + 1) * tch], psums[j][:])
            nc.sync.dma_start(y[:], y_sb[:])

    nc.compile()
    return nc


def _get_sparse_program(kc_pad, tc_pad):
    key = ("sparse", kc_pad, tc_pad)
    if key not in _prog_cache:
        _prog_cache[key] = _build_sparse_program(kc_pad, tc_pad)
    return _prog_cache[key]


def _sparse_run(flat, adj, rows, cols, trace):
    from concourse.bass_utils import run_bass_kernel_spmd

    nr, kc = len(rows), len(cols)
    per_core = -(-nr // N_CORES) if nr else 0
    tc_pad = max(P, -(-per_core // P) * P)
    kc_pad = max(P, -(-kc // P) * P)
    nstripe = kc_pad // P
    prog = _get_sparse_program(kc_pad, tc_pad)

    xsub = np.zeros((B, kc_pad), np.float32)
    if kc:
        xsub[:, :kc] = flat[:, cols]
    xs_host = np.ascontiguousarray(
        xsub.reshape(B, nstripe, P).transpose(2, 1, 0).reshape(P, nstripe * B)
    )

    in_maps = []
    row_blocks = []
    for m in range(N_CORES):
        rows_m = rows[m * per_core : (m + 1) * per_core] if per_core else rows[:0]
        row_blocks.append(rows_m)
        asb_m = np.zeros((kc_pad, tc_pad), np.float32)
        if len(rows_m) and kc:
            asb_m[:kc, : len(rows_m)] = adj[np.ix_(rows_m, cols)].T
        in_maps.append({"asb": asb_m, "xs": xs_host})

    res = run_bass_kernel_spmd(prog, in_maps, core_ids=list(range(N_CORES)), trace=trace)

    out = np.zeros((B, T), np.float32)
    for m in range(N_CORES):
        rows_m = row_blocks[m]
        if len(rows_m):
            out[:, rows_m] = res.results[m]["y"][:, : len(rows_m)]
    return out.reshape(B, 128, 128), res


# --------------------------------------------------------------------------
# dense fallback: stream the full adjacency (fp16 hi/lo split, exact)
# --------------------------------------------------------------------------

def _build_dense_program():
    import concourse.bacc as bacc
    import concourse.tile as tile
    from concourse import bass, mybir

    f16 = mybir.dt.float16
    f32 = mybir.dt.float32

    nc = bacc.Bacc("TRN2", target_bir_lowering=False, debug=False)
    adjt2 = nc.dram_tensor("adjt2", [S, 2, TBLK], f16, kind="ExternalInput").ap()
    xt = nc.dram_tensor("xt", [P, S_TILES * 2 * B], f16, kind="ExternalInput").ap()
    # rows 0-7: xh*(ah+al); rows 8-15: xl*(ah+al); folded on the host
    y2 = nc.dram_tensor("y2", [2 * B, TBLK], f32, kind="ExternalOutput").ap()

    with tile.TileContext(nc) as tc:
        with (
            tc.tile_pool(name="adj", bufs=2) as adj_pool,
            tc.tile_pool(name="misc", bufs=1) as misc_pool,
            tc.tile_pool(name="psum", bufs=1, space=bass.MemorySpace.PSUM) as psum_pool,
        ):
            xt_sb = misc_pool.tile([P, S_TILES * 2 * B], f16)
            nc.sync.dma_start(xt_sb[:], xt[:])
            y_sb = misc_pool.tile([2 * B, TBLK], f32)
            psums = [
                psum_pool.tile([2 * B, TCH_DENSE], f32, name=f"psum{j}")
                for j in range(NCH_DENSE)
            ]

            # [S, 2, TBLK] -> [P, S_TILES, 2, TBLK]: stripe i on partition p
            adjt2_r = adjt2.rearrange("(i p) h t -> p i h t", p=P)
            slabs = [G] * (S_TILES // G)
            off = 0
            for si, sz in enumerate(slabs):
                at = adj_pool.tile([P, sz, 2, TBLK], f16, name="at", tag="at")
                if si == len(slabs) - 1:
                    # final slab: per-stripe sub-DMAs into the same slot, so the
                    # PE tail after the stream ends is one stripe, not eight.
                    for g in range(sz):
                        nc.sync.dma_start(
                            at[:, g : g + 1], adjt2_r[:, off + g : off + g + 1]
                        )
                elif si == len(slabs) - 2:
                    # half-slab deps let the PE start this slab mid-DMA
                    hs = sz // 2
                    nc.sync.dma_start(at[:, 0:hs], adjt2_r[:, off : off + hs])
                    nc.sync.dma_start(at[:, hs:sz], adjt2_r[:, off + hs : off + sz])
                else:
                    nc.sync.dma_start(at[:], adjt2_r[:, off : off + sz])
                for g in range(sz):
                    i = off + g
                    lhsT = xt_sb[:, i * 2 * B : (i + 1) * 2 * B]  # [xh | xl]
                    for j in range(NCH_DENSE):
                        for h in range(2):  # moving pass over a_hi then a_lo
                            nc.tensor.matmul(
                                psums[j][:],
                                lhsT,
                                at[:, g, h, j * TCH_DENSE : (j + 1) * TCH_DENSE],
                                start=(i == 0 and h == 0),
                                stop=(i == S_TILES - 1 and h == 1),
                            )
                off += sz
            assert off == S_TILES
            for j in range(NCH_DENSE):
                nc.vector.tensor_copy(
                    y_sb[:, j * TCH_DENSE : (j + 1) * TCH_DENSE], psums[j][:]
                )
            nc.sync.dma_start(y2[:], y_sb[:])

    nc.compile()
    return nc


def _get_dense_program():
    if "dense" not in _prog_cache:
        _prog_cache["dense"] = _build_dense_program()
    return _prog_cache["dense"]


def _split16(a32):
    hi = a32.astype(np.float16)
    lo = (a32 - hi.astype(np.float32)).astype(np.float16)
    return hi, lo


def _dense_host_prep(flat, adj):
    xh, xl = _split16(flat)
    # xt[p, i*16 + h*8 + b] = x_half[h][b, i*128 + p]
    arr = np.stack([xh.reshape(B, S_TILES, P), xl.reshape(B, S_TILES, P)], axis=0)
    xt_host = np.ascontiguousarray(
        arr.transpose(3, 2, 0, 1).reshape(P, S_TILES * 2 * B)
    )

    def prep_core(m):
        blkT = np.ascontiguousarray(adj[m * TBLK : (m + 1) * TBLK, :].T)  # [S, TBLK]
        ah, al = _split16(blkT)
        adjt2_m = np.ascontiguousarray(np.stack([ah, al], axis=1))  # [S, 2, TBLK]
        return {"adjt2": adjt2_m, "xt": xt_host}

    with ThreadPoolExecutor(max_workers=N_CORES) as ex:
        in_maps = list(ex.map(prep_core, range(N_CORES)))
    return in_maps


def _dense_run(flat, adj, trace):
    from concourse.bass_utils import run_bass_kernel_spmd

    nc = _get_dense_program()
    in_maps = _dense_host_prep(flat, adj)
    res = run_bass_kernel_spmd(nc, in_maps, core_ids=list(range(N_CORES)), trace=trace)
    out = np.concatenate(
        [res.results[m]["y2"][0:B] + res.results[m]["y2"][B : 2 * B]
         for m in range(N_CORES)],
        axis=1,
    )
    return out.reshape(B, 128, 128), res


# --------------------------------------------------------------------------
# entry points
# --------------------------------------------------------------------------

def run(spikes_A, adjacency, trace=False):
    """Run on hardware; returns (out [8,128,128] f32, BassKernelResults)."""
    flat = np.ascontiguousarray(np.asarray(spikes_A, dtype=np.float32)).reshape(B, S)
    adj = np.asarray(adjacency, dtype=np.float32)

    rows = np.flatnonzero(adj.any(axis=1))
    cols = np.flatnonzero(adj.any(axis=0))
    per_core = -(-len(rows) // N_CORES) if len(rows) else 0
    tc_pad = max(P, -(-per_core // P) * P)
    kc_pad = max(P, -(-len(cols) // P) * P)
    if kc_pad <= SPARSE_MAX_KC and tc_pad <= SPARSE_MAX_TC:
        if len(rows):
            # the nonzero sub-block is small; check for one-nonzero-per-row
            block = adj[np.ix_(rows, cols)]
            nz = block != 0
            if nz.sum(axis=1).max() == 1:
                cidx = np.argmax(nz, axis=1)
                w = block[np.arange(len(rows)), cidx]
                src = cols[cidx]
                return _diag_run(flat, rows, src, w, trace)
        return _sparse_run(flat, adj, rows, cols, trace)
    return _dense_run(flat, adj, trace)


def kernel(spikes_A, adjacency):
    out, _ = run(spikes_A, adjacency)
    return out
